# revision 27
# baseline (speedup 1.0000x reference)
"""Bass/Trainium2 kernel for the single-head dense attention block.

Reference computation (per batch element b of 8):
    qkv = x @ w_qkv.T                      # [N, 3C]
    q, k, v = qkv split                    # each [N, C]
    a = softmax(q @ k.T / sqrt(C))         # [N, N]
    o = a @ v                              # [N, C]
    o2 = o.swapaxes(0,1).reshape(N, C)     # torch-faithful permutation
    out = o2 @ w_proj.T + b_proj           # [N, C]

Sharding: batch B=8 data-parallel across the 8 NeuronCores, no collectives.

Layout strategy (zero on-device transposes; host pre-transposes weights/x):
  - q.k fold:  a[n,m] = x_n^T (W_q^T W_k) x_m, so the device never computes
    q or k. Host passes hT = (W_q^T W_k)^T = W_k^T W_q in bf16; the device
    computes z = hT.T @ xT  ([c,m] layout, 1/3 the cost of qT+kT), then
    aT[m,n] = z[:,m].T @ xT[:,n].
  - v computed in [m,c] layout:      v[m,c]  = xT[:,m].T @ wvT
  - p = exp(aT/32) (no max-sub; logits are O(5) so fp32 exp is safe)
  - o in [n,c] layout:               o[n,c]  = p[:,n].T @ v
    with v augmented by a ones column so rowsum(p) lands in [n,1] per-partition
  - the torch permutation satisfies out[2t+s, d] = sum_c2 o[1024s+c2, t] *
    wprojT[c2, d], i.e. proj is a plain matmul over o's partition axis in
    half-blocks; output rows written with a stride-2 row DMA.
"""

import numpy as np
import ml_dtypes

B, N, C = 8, 2048, 1024
P = 128
NB = 512          # free-dim block for matmuls (one PSUM bank)
SCALE = 1.0 / 32.0


def _patch_tile_drain():
    """Walrus in this container rejects >~4 sem waits on one instruction; the
    TileContext exit drain aggregates one wait per active processor. Re-emit
    them as individual SP wait_ge instructions before the drain."""
    import concourse.tile as tile
    from concourse import mybir
    from concourse.vector_clock import ScopedClock

    if getattr(tile.TileContext, "_drain_patched", False):
        return

    def _drain_and_barrier(self, tick_clock, wait_clock):
        nc = self.nc
        probe = nc.sync.nop(nofuse=True)
        wait_clock.add_sem_waits(
            probe.ins, ScopedClock({None: tick_clock.global_clock})
        )
        si = probe.ins.sync_info
        waits = list(si.on_wait) if si is not None and si.on_wait else []
        probe.ins.sync_info = mybir.SyncInfo(
            on_wait=[],
            on_update=list(si.on_update) if si is not None and si.on_update else [],
        )
        handles = {h.num: h for h in self.sems.allocated().values()}
        for w in waits:
            assert w.wait_mode == "sem-ge-imm", w
            nc.sync.wait_ge(handles[w.id], w.wait_value)
        nc.sync.drain()
        nc.all_engine_barrier()
        popped = nc._tile_sem_poison_stack.pop()
        assert popped is self._sem_poison
        nc.clear_and_free_semaphores(list(self.sems.allocated().values()))
        nc.all_engine_barrier()

    tile.TileContext._drain_and_barrier = _drain_and_barrier
    tile.TileContext._drain_patched = True


def _split_excess_waits(nc, max_keep=1):
    """Walrus in this container rejects instructions with more than a couple
    of sem waits. Move excess waits onto single-wait EventSemaphore
    instructions inserted just before the offender on the same engine
    (engines execute their stream in order, so a chain of waits == one
    multi-wait)."""
    from concourse import mybir

    ctr = 0
    for f in nc.m.functions:
        for bb in f.blocks:
            il = list(bb.instructions)
            out = []
            changed = False
            for inst in il:
                si = inst.sync_info
                waits = list(si.on_wait) if si is not None and si.on_wait else []
                if len(waits) > max_keep:
                    changed = True
                    excess, keep = waits[:-max_keep], waits[-max_keep:]
                    for w in excess:
                        ev = mybir.InstEventSemaphore(
                            name=f"I-wsplit-{ctr}", ins=[], outs=[]
                        )
                        ctr += 1
                        ev.engine = inst.engine
                        ev.sync_info = mybir.SyncInfo(on_wait=[w], on_update=[])
                        out.append(ev)
                    inst.sync_info = mybir.SyncInfo(
                        on_wait=keep,
                        on_update=list(si.on_update) if si.on_update else [],
                    )
                out.append(inst)
            if changed:
                bb.instructions = out
    return nc


def build_nc(split_waits=True):
    import concourse.bass as bass
    import concourse.tile as tile
    from concourse import mybir

    _patch_tile_drain()

    bf16 = mybir.dt.bfloat16
    f32 = mybir.dt.float32

    nc = bass.Bass()
    xT_ext = nc.declare_dram_parameter("xT", [C, N], bf16, isOutput=False)
    hT_ext = nc.declare_dram_parameter("hT", [C, C], bf16, isOutput=False)
    wvT_ext = nc.declare_dram_parameter("wvT", [C, C], bf16, isOutput=False)
    wprojT_ext = nc.declare_dram_parameter("wprojT", [C, C], bf16, isOutput=False)
    bias_ext = nc.declare_dram_parameter("bias", [P, C], f32, isOutput=False)
    out_ext = nc.declare_dram_parameter("out", [N, C], f32, isOutput=True)

    CC = C // P           # 8 contraction chunks over C
    MT = N // P           # 16 m-tiles
    NBLK = N // NB        # 4 n blocks
    CB = C // NB          # 2 c blocks

    xT_r = xT_ext[:, :].rearrange("(cc p) n -> p cc n", p=P)
    hT_r = hT_ext[:, :].rearrange("(cc p) d -> p cc d", p=P)
    wvT_r = wvT_ext[:, :].rearrange("(cc p) d -> p cc d", p=P)
    wprojT_r = wprojT_ext[:, :].rearrange("(cc p) d -> p cc d", p=P)
    out_r = out_ext[:, :].rearrange("(t s) d -> t s d", s=2)

    with tile.TileContext(nc) as tc:
        with (
            tc.tile_pool(name="persist", bufs=1) as persist,
            tc.tile_pool(name="psum_main", bufs=6, space="PSUM") as psum_main,
            tc.tile_pool(name="psum_sum", bufs=2, space="PSUM") as psum_sum,
        ):
            # ---- persistent SBUF tensors ----
            z_sb = persist.tile([P, CC, N], bf16, tag="z")
            v_sb = persist.tile([P, MT, C + 1], bf16, tag="v")
            wprojT_sb = persist.tile([P, CC, C], bf16, tag="wprojT")
            bias_sb = persist.tile([P, C], f32, tag="bias")

            # ones column for the softmax denominator
            nc.vector.memset(v_sb[:, :, C : C + 1], 1.0)

            # xT stays resident through phase B (aT rhs); weights pool is
            # freed after phase A.
            with tc.tile_pool(name="xpool", bufs=1) as xpool:
                x_t = [
                    [
                        xpool.tile([P, NB], bf16, tag=f"xTt_{cc}_{nb}", name=f"xTt_{cc}_{nb}")
                        for nb in range(NBLK)
                    ]
                    for cc in range(CC)
                ]

                def dma_x(cc, nb):
                    nc.sync.dma_start(
                        out=x_t[cc][nb], in_=xT_r[:, cc, nb * NB : (nb + 1) * NB]
                    )

                # ---- phase A: z = hT.T @ xT and v = xT.T @ wvT ----
                with tc.tile_pool(name="wpool", bufs=1) as wpool:
                    h_t = [
                        [
                            wpool.tile([P, NB], bf16, tag=f"ht_{cc}_{k}", name=f"ht_{cc}_{k}")
                            for k in range(CB)
                        ]
                        for cc in range(CC)
                    ]
                    wv_t = [
                        [
                            wpool.tile([P, NB], bf16, tag=f"wvt_{cc}_{k}", name=f"wvt_{cc}_{k}")
                            for k in range(CB)
                        ]
                        for cc in range(CC)
                    ]

                    # DMA order == consumption order: first z group needs
                    # h(*,0) + x(*,0); v needs wv late.
                    # first wave: (h, x) pairs in the order z's j=0 group
                    # consumes them, spread across engines' DGE rings
                    first_engines = [nc.sync, nc.scalar, nc.sync, nc.scalar]
                    for cc in range(CC):
                        e0 = first_engines[(2 * cc) % 4]
                        e1 = first_engines[(2 * cc + 1) % 4]
                        e0.dma_start(out=h_t[cc][0], in_=hT_r[:, cc, 0:NB])
                        e1.dma_start(
                            out=x_t[cc][0], in_=xT_r[:, cc, 0:NB]
                        )
                    for nb in range(1, NBLK):
                        for cc in range(CC):
                            dma_x(cc, nb)
                    for cc in range(CC):
                        nc.sync.dma_start(
                            out=h_t[cc][1], in_=hT_r[:, cc, NB : 2 * NB]
                        )
                    for k in range(CB):
                        for cc in range(CC):
                            nc.sync.dma_start(
                                out=wv_t[cc][k], in_=wvT_r[:, cc, k * NB : (k + 1) * NB]
                            )
                    # needed only at the tail — load after the critical inputs
                    nc.sync.dma_start(out=bias_sb, in_=bias_ext[:, :])
                    for cc in range(CC):
                        nc.sync.dma_start(
                            out=wprojT_sb[:, cc, :], in_=wprojT_r[:, cc, :]
                        )

                    # z[g-tile, n-block] = hT[:, g].T @ xT[:, n]
                    for j in range(CC):
                        psums = [psum_main.tile([P, NB], f32, tag="ps", name=f"ps_z_{j}_{i}") for i in range(NBLK)]
                        for cc in range(CC):
                            lhsT = h_t[cc][j // 4][:, (j % 4) * P : (j % 4 + 1) * P]
                            for nb in range(NBLK):
                                nc.tensor.matmul(
                                    psums[nb],
                                    lhsT,
                                    x_t[cc][nb],
                                    start=(cc == 0),
                                    stop=(cc == CC - 1),
                                )
                        for nb in range(NBLK):
                            nc.vector.tensor_copy(
                                out=z_sb[:, j, nb * NB : (nb + 1) * NB], in_=psums[nb]
                            )

                    # v[m-tile, c-block] = xT[:, m].T @ wvT[:, c]
                    for mt in range(MT):
                        psums = [psum_main.tile([P, NB], f32, tag="ps", name=f"ps_v_{mt}_{i}") for i in range(CB)]
                        for cc in range(CC):
                            lhsT = x_t[cc][mt // 4][:, (mt % 4) * P : (mt % 4 + 1) * P]
                            for cb in range(CB):
                                nc.tensor.matmul(
                                    psums[cb],
                                    lhsT,
                                    wv_t[cc][cb],
                                    start=(cc == 0),
                                    stop=(cc == CC - 1),
                                )
                        for cb in range(CB):
                            nc.scalar.activation(
                                out=v_sb[:, mt, cb * NB : (cb + 1) * NB],
                                in_=psums[cb],
                                func=mybir.ActivationFunctionType.Copy,
                            )

                # ---- phase B: attention, one 512-wide n-block at a time ----
                with (
                    tc.tile_pool(name="attn", bufs=1) as attn_pool,
                    tc.tile_pool(name="pT", bufs=1) as pT_pool,
                    tc.tile_pool(name="small", bufs=8) as small_pool,
                    tc.tile_pool(name="outbuf", bufs=6) as out_pool,
                ):
                    o_sb = attn_pool.tile([P, MT, C], bf16, tag="o")
                    SB = 2  # n-blocks per superblock: one aT weight load
                    #         (z m-slice) feeds SB matmuls
                    for sbk in range(NBLK // SB):
                        pT = pT_pool.tile([P, MT, SB * NB], bf16, tag="pT")
                        # aT[m-tile, nblk] = z[:, m].T @ xT[:, nblk]; p = exp(aT/32)
                        for mt in range(MT):
                            apsums = [
                                psum_main.tile([P, NB], f32, tag="ps", name=f"ps_a_{sbk}_{mt}_{u}")
                                for u in range(SB)
                            ]
                            for cc in range(CC):
                                lhsT = z_sb[:, cc, mt * P : (mt + 1) * P]
                                for u in range(SB):
                                    nc.tensor.matmul(
                                        apsums[u],
                                        lhsT,
                                        x_t[cc][sbk * SB + u],
                                        start=(cc == 0),
                                        stop=(cc == CC - 1),
                                    )
                            for u in range(SB):
                                nc.scalar.activation(
                                    out=pT[:, mt, u * NB : (u + 1) * NB],
                                    in_=apsums[u],
                                    func=mybir.ActivationFunctionType.Exp,
                                    scale=SCALE,
                                )
                        # o[n-tile, c] = p[:, n].T @ v  (+ ones column -> rowsum)
                        for j in range(SB * NB // P):
                            nt = sbk * (SB * NB // P) + j
                            opsums = [psum_main.tile([P, NB], f32, tag="ps", name=f"ps_o_{nt}_{i}") for i in range(CB)]
                            osum = psum_sum.tile([P, 1], f32, tag="ps_sum", name=f"ps_sum_{nt}")
                            for mt in range(MT):
                                lhsT = pT[:, mt, j * P : (j + 1) * P]
                                for cb in range(CB):
                                    nc.tensor.matmul(
                                        opsums[cb],
                                        lhsT,
                                        v_sb[:, mt, cb * NB : (cb + 1) * NB],
                                        start=(mt == 0),
                                        stop=(mt == MT - 1),
                                    )
                                nc.tensor.matmul(
                                    osum,
                                    lhsT,
                                    v_sb[:, mt, C : C + 1],
                                    start=(mt == 0),
                                    stop=(mt == MT - 1),
                                )
                            recip = small_pool.tile([P, 1], f32, tag="recip")
                            nc.vector.reciprocal(out=recip, in_=osum)
                            for cb in range(CB):
                                nc.vector.tensor_scalar_mul(
                                    out=o_sb[:, nt, cb * NB : (cb + 1) * NB],
                                    in0=opsums[cb],
                                    scalar1=recip,
                                )

                    # ---- phase C: out[2t+s, d] = sum_c2 o[1024s+c2, t] wprojT[c2, d]
                    for s in range(2):
                        for tt in range(CC):  # 8 t-tiles of 128 (t in [0,1024))
                            psums = [psum_main.tile([P, NB], f32, tag="ps", name=f"ps_p_{s}_{tt}_{i}") for i in range(CB)]
                            for k in range(CC):
                                lhsT = o_sb[:, CC * s + k, tt * P : (tt + 1) * P]
                                for db in range(CB):
                                    nc.tensor.matmul(
                                        psums[db],
                                        lhsT,
                                        wprojT_sb[:, k, db * NB : (db + 1) * NB],
                                        start=(k == 0),
                                        stop=(k == CC - 1),
                                    )
                            for db in range(CB):
                                outt = out_pool.tile([P, NB], f32, tag="outt", name=f"outt_{s}_{tt}_{db}")
                                nc.vector.tensor_add(
                                    out=outt,
                                    in0=psums[db],
                                    in1=bias_sb[:, db * NB : (db + 1) * NB],
                                )
                                nc.sync.dma_start(
                                    out=out_r[
                                        tt * P : (tt + 1) * P, s, db * NB : (db + 1) * NB
                                    ],
                                    in_=outt,
                                )
    if split_waits:
        _split_excess_waits(nc)
    return nc


_CACHED_NC = None


def _get_nc():
    global _CACHED_NC
    if _CACHED_NC is None:
        _CACHED_NC = build_nc()
    return _CACHED_NC


def _make_in_maps(x, w_qkv, w_proj, b_proj):
    bf16 = ml_dtypes.bfloat16
    x = np.asarray(x, dtype=np.float32)
    w_qkv = np.asarray(w_qkv, dtype=np.float32)
    w_proj = np.asarray(w_proj, dtype=np.float32)
    b_proj = np.asarray(b_proj, dtype=np.float32)

    w_q, w_k, w_v = w_qkv[0:C], w_qkv[C : 2 * C], w_qkv[2 * C : 3 * C]
    # hT = (W_q^T W_k)^T = W_k^T W_q, computed in f32 then rounded once
    hT = np.ascontiguousarray(w_k.T @ w_q).astype(bf16)
    wvT = np.ascontiguousarray(w_v.T).astype(bf16)
    wprojT = np.ascontiguousarray(w_proj.T).astype(bf16)
    bias = np.ascontiguousarray(np.broadcast_to(b_proj, (P, C)))
    in_maps = []
    for b in range(B):
        xT = np.ascontiguousarray(x[b].T).astype(bf16)
        in_maps.append(
            {"xT": xT, "hT": hT, "wvT": wvT, "wprojT": wprojT, "bias": bias}
        )
    return in_maps


def kernel(x, w_qkv, w_proj, b_proj):
    from concourse.bass_utils import run_bass_kernel_spmd

    nc = _get_nc()
    in_maps = _make_in_maps(x, w_qkv, w_proj, b_proj)
    res = run_bass_kernel_spmd(nc, in_maps, core_ids=list(range(B)))
    return np.stack([res.results[b]["out"] for b in range(B)], axis=0)


def kernel_traced(x, w_qkv, w_proj, b_proj, **trace_kwargs):
    """Like kernel() but with NTFF profiling; returns (out, BassKernelResults)."""
    from concourse.bass_utils import run_bass_kernel_spmd

    nc = _get_nc()
    in_maps = _make_in_maps(x, w_qkv, w_proj, b_proj)
    res = run_bass_kernel_spmd(
        nc, in_maps, core_ids=list(range(B)), trace=True, **trace_kwargs
    )
    out = np.stack([res.results[b]["out"] for b in range(B)], axis=0)
    return out, res


# revision 28
# speedup vs baseline: 1.1891x; 1.1891x over previous
"""Bass/Trainium2 kernel for the single-head dense attention block.

Reference computation (per batch element b of 8):
    qkv = x @ w_qkv.T                      # [N, 3C]
    q, k, v = qkv split                    # each [N, C]
    a = softmax(q @ k.T / sqrt(C))         # [N, N]
    o = a @ v                              # [N, C]
    o2 = o.swapaxes(0,1).reshape(N, C)     # torch-faithful permutation
    out = o2 @ w_proj.T + b_proj           # [N, C]

Sharding: batch B=8 data-parallel across the 8 NeuronCores, no collectives.

Layout strategy (zero on-device transposes; host pre-transposes weights/x):
  - q.k fold:  a[n,m] = x_n^T (W_q^T W_k) x_m, so the device never computes
    q or k. Host passes hT = (W_q^T W_k)^T = W_k^T W_q in bf16; the device
    computes z = hT.T @ xT  ([c,m] layout, 1/3 the cost of qT+kT), then
    aT[m,n] = z[:,m].T @ xT[:,n].
  - v computed in [m,c] layout:      v[m,c]  = xT[:,m].T @ wvT
  - p = exp(aT/32) (no max-sub; logits are O(5) so fp32 exp is safe)
  - o in [n,c] layout:               o[n,c]  = p[:,n].T @ v
    with v augmented by a ones column so rowsum(p) lands in [n,1] per-partition
  - the torch permutation satisfies out[2t+s, d] = sum_c2 o[1024s+c2, t] *
    wprojT[c2, d], i.e. proj is a plain matmul over o's partition axis in
    half-blocks; output rows written with a stride-2 row DMA.
"""

import numpy as np
import ml_dtypes

B, N, C = 8, 2048, 1024
P = 128
NB = 512          # free-dim block for matmuls (one PSUM bank)
SCALE = 1.0 / 32.0


def _patch_tile_drain():
    """Walrus in this container rejects >~4 sem waits on one instruction; the
    TileContext exit drain aggregates one wait per active processor. Re-emit
    them as individual SP wait_ge instructions before the drain."""
    import concourse.tile as tile
    from concourse import mybir
    from concourse.vector_clock import ScopedClock

    if getattr(tile.TileContext, "_drain_patched", False):
        return

    def _drain_and_barrier(self, tick_clock, wait_clock):
        nc = self.nc
        probe = nc.sync.nop(nofuse=True)
        wait_clock.add_sem_waits(
            probe.ins, ScopedClock({None: tick_clock.global_clock})
        )
        si = probe.ins.sync_info
        waits = list(si.on_wait) if si is not None and si.on_wait else []
        probe.ins.sync_info = mybir.SyncInfo(
            on_wait=[],
            on_update=list(si.on_update) if si is not None and si.on_update else [],
        )
        handles = {h.num: h for h in self.sems.allocated().values()}
        for w in waits:
            assert w.wait_mode == "sem-ge-imm", w
            nc.sync.wait_ge(handles[w.id], w.wait_value)
        nc.sync.drain()
        nc.all_engine_barrier()
        popped = nc._tile_sem_poison_stack.pop()
        assert popped is self._sem_poison
        nc.clear_and_free_semaphores(list(self.sems.allocated().values()))
        nc.all_engine_barrier()

    tile.TileContext._drain_and_barrier = _drain_and_barrier
    tile.TileContext._drain_patched = True


def _split_excess_waits(nc, max_keep=1):
    """Walrus in this container rejects instructions with more than a couple
    of sem waits. Move excess waits onto single-wait EventSemaphore
    instructions inserted just before the offender on the same engine
    (engines execute their stream in order, so a chain of waits == one
    multi-wait)."""
    from concourse import mybir

    ctr = 0
    for f in nc.m.functions:
        for bb in f.blocks:
            il = list(bb.instructions)
            out = []
            changed = False
            for inst in il:
                si = inst.sync_info
                waits = list(si.on_wait) if si is not None and si.on_wait else []
                if len(waits) > max_keep:
                    changed = True
                    excess, keep = waits[:-max_keep], waits[-max_keep:]
                    for w in excess:
                        ev = mybir.InstEventSemaphore(
                            name=f"I-wsplit-{ctr}", ins=[], outs=[]
                        )
                        ctr += 1
                        ev.engine = inst.engine
                        ev.sync_info = mybir.SyncInfo(on_wait=[w], on_update=[])
                        out.append(ev)
                    inst.sync_info = mybir.SyncInfo(
                        on_wait=keep,
                        on_update=list(si.on_update) if si.on_update else [],
                    )
                out.append(inst)
            if changed:
                bb.instructions = out
    return nc


def build_nc(split_waits=True):
    import concourse.bass as bass
    import concourse.tile as tile
    from concourse import mybir

    _patch_tile_drain()

    bf16 = mybir.dt.bfloat16
    f32 = mybir.dt.float32

    nc = bass.Bass()
    xT_ext = nc.declare_dram_parameter("xT", [C, N], bf16, isOutput=False)
    hT_ext = nc.declare_dram_parameter("hT", [C, C], bf16, isOutput=False)
    wvT_ext = nc.declare_dram_parameter("wvT", [C, C], bf16, isOutput=False)
    wprojT_ext = nc.declare_dram_parameter("wprojT", [C, C], bf16, isOutput=False)
    bias_ext = nc.declare_dram_parameter("bias", [P, C], f32, isOutput=False)
    out_ext = nc.declare_dram_parameter("out", [N, C], f32, isOutput=True)

    CC = C // P           # 8 contraction chunks over C
    MT = N // P           # 16 m-tiles
    NBLK = N // NB        # 4 n blocks
    CB = C // NB          # 2 c blocks

    xT_r = xT_ext[:, :].rearrange("(cc p) n -> p cc n", p=P)
    hT_r = hT_ext[:, :].rearrange("(cc p) d -> p cc d", p=P)
    wvT_r = wvT_ext[:, :].rearrange("(cc p) d -> p cc d", p=P)
    wprojT_r = wprojT_ext[:, :].rearrange("(cc p) d -> p cc d", p=P)
    out_r = out_ext[:, :].rearrange("(t s) d -> t s d", s=2)

    with tile.TileContext(nc) as tc:
        with (
            tc.tile_pool(name="persist", bufs=1) as persist,
            tc.tile_pool(name="psum_main", bufs=6, space="PSUM") as psum_main,
            tc.tile_pool(name="psum_sum", bufs=2, space="PSUM") as psum_sum,
        ):
            # ---- persistent SBUF tensors ----
            z_sb = persist.tile([P, CC, N], bf16, tag="z")
            v_sb = persist.tile([P, MT, C + 1], bf16, tag="v")
            wprojT_sb = persist.tile([P, CC, C], bf16, tag="wprojT")
            bias_sb = persist.tile([P, C], f32, tag="bias")

            # ones column for the softmax denominator
            nc.vector.memset(v_sb[:, :, C : C + 1], 1.0)

            # xT stays resident through phase B (aT rhs); weights pool is
            # freed after phase A.
            with tc.tile_pool(name="xpool", bufs=1) as xpool:
                x_t = [
                    [
                        xpool.tile([P, NB], bf16, tag=f"xTt_{cc}_{nb}", name=f"xTt_{cc}_{nb}")
                        for nb in range(NBLK)
                    ]
                    for cc in range(CC)
                ]

                def dma_x(cc, nb):
                    nc.sync.dma_start(
                        out=x_t[cc][nb], in_=xT_r[:, cc, nb * NB : (nb + 1) * NB]
                    )

                # ---- phase A: z = hT.T @ xT and v = xT.T @ wvT ----
                with tc.tile_pool(name="wpool", bufs=1) as wpool:
                    h_t = [
                        [
                            wpool.tile([P, NB], bf16, tag=f"ht_{cc}_{k}", name=f"ht_{cc}_{k}")
                            for k in range(CB)
                        ]
                        for cc in range(CC)
                    ]
                    wv_t = [
                        [
                            wpool.tile([P, NB], bf16, tag=f"wvt_{cc}_{k}", name=f"wvt_{cc}_{k}")
                            for k in range(CB)
                        ]
                        for cc in range(CC)
                    ]

                    # DMA order == consumption order: first z group needs
                    # h(*,0) + x(*,0); v needs wv late.
                    # first wave: (h, x) pairs in the order z's j=0 group
                    # consumes them, spread across engines' DGE rings
                    first_engines = [nc.sync, nc.scalar, nc.sync, nc.scalar]
                    for cc in range(CC):
                        e0 = first_engines[(2 * cc) % 4]
                        e1 = first_engines[(2 * cc + 1) % 4]
                        e0.dma_start(out=h_t[cc][0], in_=hT_r[:, cc, 0:NB])
                        e1.dma_start(
                            out=x_t[cc][0], in_=xT_r[:, cc, 0:NB]
                        )
                    for nb in range(1, NBLK):
                        for cc in range(CC):
                            dma_x(cc, nb)
                    for cc in range(CC):
                        nc.sync.dma_start(
                            out=h_t[cc][1], in_=hT_r[:, cc, NB : 2 * NB]
                        )
                    for k in range(CB):
                        for cc in range(CC):
                            nc.sync.dma_start(
                                out=wv_t[cc][k], in_=wvT_r[:, cc, k * NB : (k + 1) * NB]
                            )
                    # needed only at the tail — load after the critical inputs
                    nc.sync.dma_start(out=bias_sb, in_=bias_ext[:, :])
                    for cc in range(CC):
                        nc.sync.dma_start(
                            out=wprojT_sb[:, cc, :], in_=wprojT_r[:, cc, :]
                        )

                    # z[g-tile, n-block] = hT[:, g].T @ xT[:, n]
                    for j in range(CC):
                        psums = [psum_main.tile([P, NB], f32, tag="ps", name=f"ps_z_{j}_{i}") for i in range(NBLK)]
                        for cc in range(CC):
                            lhsT = h_t[cc][j // 4][:, (j % 4) * P : (j % 4 + 1) * P]
                            for nb in range(NBLK):
                                nc.tensor.matmul(
                                    psums[nb],
                                    lhsT,
                                    x_t[cc][nb],
                                    start=(cc == 0),
                                    stop=(cc == CC - 1),
                                )
                        for nb in range(NBLK):
                            nc.vector.tensor_copy(
                                out=z_sb[:, j, nb * NB : (nb + 1) * NB], in_=psums[nb]
                            )

                    # v[m-tile, c-block] = xT[:, m].T @ wvT[:, c]
                    for mt in range(MT):
                        psums = [psum_main.tile([P, NB], f32, tag="ps", name=f"ps_v_{mt}_{i}") for i in range(CB)]
                        for cc in range(CC):
                            lhsT = x_t[cc][mt // 4][:, (mt % 4) * P : (mt % 4 + 1) * P]
                            for cb in range(CB):
                                nc.tensor.matmul(
                                    psums[cb],
                                    lhsT,
                                    wv_t[cc][cb],
                                    start=(cc == 0),
                                    stop=(cc == CC - 1),
                                )
                        for cb in range(CB):
                            nc.scalar.activation(
                                out=v_sb[:, mt, cb * NB : (cb + 1) * NB],
                                in_=psums[cb],
                                func=mybir.ActivationFunctionType.Copy,
                            )

                # ---- phase B: attention, one 512-wide n-block at a time ----
                with (
                    tc.tile_pool(name="attn", bufs=1) as attn_pool,
                    tc.tile_pool(name="pT", bufs=1) as pT_pool,
                    tc.tile_pool(name="small", bufs=8) as small_pool,
                    tc.tile_pool(name="outbuf", bufs=4) as out_pool,
                ):
                    o_sb = attn_pool.tile([P, MT, C], bf16, tag="o")
                    SB = 2  # n-blocks per superblock: one aT weight load
                    #         (z m-slice) feeds SB matmuls
                    for sbk in range(NBLK // SB):
                        pT = pT_pool.tile([P, MT, SB * NB], bf16, tag="pT")
                        # aT[m-tile, nblk] = z[:, m].T @ xT[:, nblk]; p = exp(aT/32)
                        for mt in range(MT):
                            apsums = [
                                psum_main.tile([P, NB], f32, tag="ps", name=f"ps_a_{sbk}_{mt}_{u}")
                                for u in range(SB)
                            ]
                            for cc in range(CC):
                                lhsT = z_sb[:, cc, mt * P : (mt + 1) * P]
                                for u in range(SB):
                                    nc.tensor.matmul(
                                        apsums[u],
                                        lhsT,
                                        x_t[cc][sbk * SB + u],
                                        start=(cc == 0),
                                        stop=(cc == CC - 1),
                                    )
                            for u in range(SB):
                                nc.scalar.activation(
                                    out=pT[:, mt, u * NB : (u + 1) * NB],
                                    in_=apsums[u],
                                    func=mybir.ActivationFunctionType.Exp,
                                    scale=SCALE,
                                )
                        # o[n-tile, c] = p[:, n].T @ v  (+ ones column -> rowsum)
                        for j in range(SB * NB // P):
                            nt = sbk * (SB * NB // P) + j
                            opsums = [psum_main.tile([P, NB], f32, tag="ps", name=f"ps_o_{nt}_{i}") for i in range(CB)]
                            osum = psum_sum.tile([P, 1], f32, tag="ps_sum", name=f"ps_sum_{nt}")
                            for mt in range(MT):
                                lhsT = pT[:, mt, j * P : (j + 1) * P]
                                for cb in range(CB):
                                    nc.tensor.matmul(
                                        opsums[cb],
                                        lhsT,
                                        v_sb[:, mt, cb * NB : (cb + 1) * NB],
                                        start=(mt == 0),
                                        stop=(mt == MT - 1),
                                    )
                                nc.tensor.matmul(
                                    osum,
                                    lhsT,
                                    v_sb[:, mt, C : C + 1],
                                    start=(mt == 0),
                                    stop=(mt == MT - 1),
                                )
                            recip = small_pool.tile([P, 1], f32, tag="recip")
                            nc.vector.reciprocal(out=recip, in_=osum)
                            for cb in range(CB):
                                nc.vector.tensor_scalar_mul(
                                    out=o_sb[:, nt, cb * NB : (cb + 1) * NB],
                                    in0=opsums[cb],
                                    scalar1=recip,
                                )

                    # ---- phase C: out[2t+s, d] = sum_c2 o[1024s+c2, t] wprojT[c2, d]
                    for s in range(2):
                        for tt in range(CC):  # 8 t-tiles of 128 (t in [0,1024))
                            psums = [psum_main.tile([P, NB], f32, tag="ps", name=f"ps_p_{s}_{tt}_{i}") for i in range(CB)]
                            for k in range(CC):
                                lhsT = o_sb[:, CC * s + k, tt * P : (tt + 1) * P]
                                for db in range(CB):
                                    nc.tensor.matmul(
                                        psums[db],
                                        lhsT,
                                        wprojT_sb[:, k, db * NB : (db + 1) * NB],
                                        start=(k == 0),
                                        stop=(k == CC - 1),
                                    )
                            for db in range(CB):
                                outt = out_pool.tile([P, NB], f32, tag="outt", name=f"outt_{s}_{tt}_{db}")
                                nc.vector.tensor_add(
                                    out=outt,
                                    in0=psums[db],
                                    in1=bias_sb[:, db * NB : (db + 1) * NB],
                                )
                                nc.sync.dma_start(
                                    out=out_r[
                                        tt * P : (tt + 1) * P, s, db * NB : (db + 1) * NB
                                    ],
                                    in_=outt,
                                )
    if split_waits:
        _split_excess_waits(nc)
    return nc


_CACHED_NC = None


def _get_nc():
    global _CACHED_NC
    if _CACHED_NC is None:
        _CACHED_NC = build_nc()
    return _CACHED_NC


def _make_in_maps(x, w_qkv, w_proj, b_proj):
    bf16 = ml_dtypes.bfloat16
    x = np.asarray(x, dtype=np.float32)
    w_qkv = np.asarray(w_qkv, dtype=np.float32)
    w_proj = np.asarray(w_proj, dtype=np.float32)
    b_proj = np.asarray(b_proj, dtype=np.float32)

    w_q, w_k, w_v = w_qkv[0:C], w_qkv[C : 2 * C], w_qkv[2 * C : 3 * C]
    # hT = (W_q^T W_k)^T = W_k^T W_q, computed in f32 then rounded once
    hT = np.ascontiguousarray(w_k.T @ w_q).astype(bf16)
    wvT = np.ascontiguousarray(w_v.T).astype(bf16)
    wprojT = np.ascontiguousarray(w_proj.T).astype(bf16)
    bias = np.ascontiguousarray(np.broadcast_to(b_proj, (P, C)))
    in_maps = []
    for b in range(B):
        xT = np.ascontiguousarray(x[b].T).astype(bf16)
        in_maps.append(
            {"xT": xT, "hT": hT, "wvT": wvT, "wprojT": wprojT, "bias": bias}
        )
    return in_maps


def kernel(x, w_qkv, w_proj, b_proj):
    from concourse.bass_utils import run_bass_kernel_spmd

    nc = _get_nc()
    in_maps = _make_in_maps(x, w_qkv, w_proj, b_proj)
    res = run_bass_kernel_spmd(nc, in_maps, core_ids=list(range(B)))
    return np.stack([res.results[b]["out"] for b in range(B)], axis=0)


def kernel_traced(x, w_qkv, w_proj, b_proj, **trace_kwargs):
    """Like kernel() but with NTFF profiling; returns (out, BassKernelResults)."""
    from concourse.bass_utils import run_bass_kernel_spmd

    nc = _get_nc()
    in_maps = _make_in_maps(x, w_qkv, w_proj, b_proj)
    res = run_bass_kernel_spmd(
        nc, in_maps, core_ids=list(range(B)), trace=True, **trace_kwargs
    )
    out = np.stack([res.results[b]["out"] for b in range(B)], axis=0)
    return out, res


# revision 30
# speedup vs baseline: 1.1910x; 1.0016x over previous
"""Bass/Trainium2 kernel for the single-head dense attention block.

Reference computation (per batch element b of 8):
    qkv = x @ w_qkv.T                      # [N, 3C]
    q, k, v = qkv split                    # each [N, C]
    a = softmax(q @ k.T / sqrt(C))         # [N, N]
    o = a @ v                              # [N, C]
    o2 = o.swapaxes(0,1).reshape(N, C)     # torch-faithful permutation
    out = o2 @ w_proj.T + b_proj           # [N, C]

Sharding: batch B=8 data-parallel across the 8 NeuronCores, no collectives.

Layout strategy (zero on-device transposes; host pre-transposes weights/x):
  - q.k fold:  a[n,m] = x_n^T (W_q^T W_k) x_m, so the device never computes
    q or k. Host passes hT = (W_q^T W_k)^T = W_k^T W_q in bf16; the device
    computes z = hT.T @ xT  ([c,m] layout, 1/3 the cost of qT+kT), then
    aT[m,n] = z[:,m].T @ xT[:,n].
  - v computed in [m,c] layout:      v[m,c]  = xT[:,m].T @ wvT
  - p = exp(aT/32) (no max-sub; logits are O(5) so fp32 exp is safe)
  - o in [n,c] layout:               o[n,c]  = p[:,n].T @ v
    with v augmented by a ones column so rowsum(p) lands in [n,1] per-partition
  - the torch permutation satisfies out[2t+s, d] = sum_c2 o[1024s+c2, t] *
    wprojT[c2, d], i.e. proj is a plain matmul over o's partition axis in
    half-blocks; output rows written with a stride-2 row DMA.
"""

import numpy as np
import ml_dtypes

B, N, C = 8, 2048, 1024
P = 128
NB = 512          # free-dim block for matmuls (one PSUM bank)
SCALE = 1.0 / 32.0


def _patch_tile_drain():
    """Walrus in this container rejects >~4 sem waits on one instruction; the
    TileContext exit drain aggregates one wait per active processor. Re-emit
    them as individual SP wait_ge instructions before the drain."""
    import concourse.tile as tile
    from concourse import mybir
    from concourse.vector_clock import ScopedClock

    if getattr(tile.TileContext, "_drain_patched", False):
        return

    def _drain_and_barrier(self, tick_clock, wait_clock):
        nc = self.nc
        probe = nc.sync.nop(nofuse=True)
        wait_clock.add_sem_waits(
            probe.ins, ScopedClock({None: tick_clock.global_clock})
        )
        si = probe.ins.sync_info
        waits = list(si.on_wait) if si is not None and si.on_wait else []
        probe.ins.sync_info = mybir.SyncInfo(
            on_wait=[],
            on_update=list(si.on_update) if si is not None and si.on_update else [],
        )
        handles = {h.num: h for h in self.sems.allocated().values()}
        for w in waits:
            assert w.wait_mode == "sem-ge-imm", w
            nc.sync.wait_ge(handles[w.id], w.wait_value)
        nc.sync.drain()
        nc.all_engine_barrier()
        popped = nc._tile_sem_poison_stack.pop()
        assert popped is self._sem_poison
        nc.clear_and_free_semaphores(list(self.sems.allocated().values()))
        nc.all_engine_barrier()

    tile.TileContext._drain_and_barrier = _drain_and_barrier
    tile.TileContext._drain_patched = True


def _split_excess_waits(nc, max_keep=1):
    """Walrus in this container rejects instructions with more than a couple
    of sem waits. Move excess waits onto single-wait EventSemaphore
    instructions inserted just before the offender on the same engine
    (engines execute their stream in order, so a chain of waits == one
    multi-wait)."""
    from concourse import mybir

    ctr = 0
    for f in nc.m.functions:
        for bb in f.blocks:
            il = list(bb.instructions)
            out = []
            changed = False
            for inst in il:
                si = inst.sync_info
                waits = list(si.on_wait) if si is not None and si.on_wait else []
                if len(waits) > max_keep:
                    changed = True
                    excess, keep = waits[:-max_keep], waits[-max_keep:]
                    for w in excess:
                        ev = mybir.InstEventSemaphore(
                            name=f"I-wsplit-{ctr}", ins=[], outs=[]
                        )
                        ctr += 1
                        ev.engine = inst.engine
                        ev.sync_info = mybir.SyncInfo(on_wait=[w], on_update=[])
                        out.append(ev)
                    inst.sync_info = mybir.SyncInfo(
                        on_wait=keep,
                        on_update=list(si.on_update) if si.on_update else [],
                    )
                out.append(inst)
            if changed:
                bb.instructions = out
    return nc


def build_nc(split_waits=True):
    import concourse.bass as bass
    import concourse.tile as tile
    from concourse import mybir

    _patch_tile_drain()

    bf16 = mybir.dt.bfloat16
    f32 = mybir.dt.float32

    nc = bass.Bass()
    xT_ext = nc.declare_dram_parameter("xT", [C, N], bf16, isOutput=False)
    hT_ext = nc.declare_dram_parameter("hT", [C, C], bf16, isOutput=False)
    wvT_ext = nc.declare_dram_parameter("wvT", [C, C], bf16, isOutput=False)
    wprojT_ext = nc.declare_dram_parameter("wprojT", [C, C], bf16, isOutput=False)
    bias_ext = nc.declare_dram_parameter("bias", [P, C], f32, isOutput=False)
    out_ext = nc.declare_dram_parameter("out", [N, C], f32, isOutput=True)

    CC = C // P           # 8 contraction chunks over C
    MT = N // P           # 16 m-tiles
    NBLK = N // NB        # 4 n blocks
    CB = C // NB          # 2 c blocks

    xT_r = xT_ext[:, :].rearrange("(cc p) n -> p cc n", p=P)
    hT_r = hT_ext[:, :].rearrange("(cc p) d -> p cc d", p=P)
    wvT_r = wvT_ext[:, :].rearrange("(cc p) d -> p cc d", p=P)
    wprojT_r = wprojT_ext[:, :].rearrange("(cc p) d -> p cc d", p=P)
    out_r = out_ext[:, :].rearrange("(t s) d -> t s d", s=2)

    with tile.TileContext(nc) as tc:
        with (
            tc.tile_pool(name="persist", bufs=1) as persist,
            tc.tile_pool(name="psum_main", bufs=6, space="PSUM") as psum_main,
            tc.tile_pool(name="psum_sum", bufs=2, space="PSUM") as psum_sum,
        ):
            # ---- persistent SBUF tensors ----
            z_sb = persist.tile([P, CC, N], bf16, tag="z")
            v_sb = persist.tile([P, MT, C + 1], bf16, tag="v")
            wprojT_sb = persist.tile([P, CC, C], bf16, tag="wprojT")
            bias_sb = persist.tile([P, C], f32, tag="bias")

            # ones column for the softmax denominator
            nc.vector.memset(v_sb[:, :, C : C + 1], 1.0)

            # xT stays resident through phase B (aT rhs); weights pool is
            # freed after phase A.
            with tc.tile_pool(name="xpool", bufs=1) as xpool:
                x_t = [
                    [
                        xpool.tile([P, NB], bf16, tag=f"xTt_{cc}_{nb}", name=f"xTt_{cc}_{nb}")
                        for nb in range(NBLK)
                    ]
                    for cc in range(CC)
                ]

                def dma_x(cc, nb):
                    nc.sync.dma_start(
                        out=x_t[cc][nb], in_=xT_r[:, cc, nb * NB : (nb + 1) * NB]
                    )

                # ---- phase A: z = hT.T @ xT and v = xT.T @ wvT ----
                with tc.tile_pool(name="wpool", bufs=1) as wpool:
                    h_t = [
                        [
                            wpool.tile([P, NB], bf16, tag=f"ht_{cc}_{k}", name=f"ht_{cc}_{k}")
                            for k in range(CB)
                        ]
                        for cc in range(CC)
                    ]
                    wv_t = [
                        [
                            wpool.tile([P, NB], bf16, tag=f"wvt_{cc}_{k}", name=f"wvt_{cc}_{k}")
                            for k in range(CB)
                        ]
                        for cc in range(CC)
                    ]

                    # DMA order == consumption order: first z group needs
                    # h(*,0) + x(*,0); v needs wv late.
                    # first wave: (h, x) pairs in the order z's j=0 group
                    # consumes them, spread across engines' DGE rings
                    first_engines = [nc.sync, nc.scalar, nc.sync, nc.scalar]
                    for cc in range(CC):
                        e0 = first_engines[(2 * cc) % 4]
                        e1 = first_engines[(2 * cc + 1) % 4]
                        e0.dma_start(out=h_t[cc][0], in_=hT_r[:, cc, 0:NB])
                        e1.dma_start(
                            out=x_t[cc][0], in_=xT_r[:, cc, 0:NB]
                        )
                    for nb in range(1, NBLK):
                        for cc in range(CC):
                            dma_x(cc, nb)
                    for cc in range(CC):
                        nc.sync.dma_start(
                            out=h_t[cc][1], in_=hT_r[:, cc, NB : 2 * NB]
                        )
                    for k in range(CB):
                        for cc in range(CC):
                            nc.sync.dma_start(
                                out=wv_t[cc][k], in_=wvT_r[:, cc, k * NB : (k + 1) * NB]
                            )
                    # needed only at the tail — load after the critical inputs
                    nc.sync.dma_start(out=bias_sb, in_=bias_ext[:, :])
                    for cc in range(CC):
                        nc.sync.dma_start(
                            out=wprojT_sb[:, cc, :], in_=wprojT_r[:, cc, :]
                        )

                    # z[g-tile, n-block] = hT[:, g].T @ xT[:, n]
                    for j in range(CC):
                        psums = [psum_main.tile([P, NB], f32, tag="ps", name=f"ps_z_{j}_{i}") for i in range(NBLK)]
                        for cc in range(CC):
                            lhsT = h_t[cc][j // 4][:, (j % 4) * P : (j % 4 + 1) * P]
                            for nb in range(NBLK):
                                nc.tensor.matmul(
                                    psums[nb],
                                    lhsT,
                                    x_t[cc][nb],
                                    start=(cc == 0),
                                    stop=(cc == CC - 1),
                                )
                        for nb in range(NBLK):
                            nc.vector.tensor_copy(
                                out=z_sb[:, j, nb * NB : (nb + 1) * NB], in_=psums[nb]
                            )

                    # v[m-tile, c-block] = xT[:, m].T @ wvT[:, c]
                    for mt in range(MT):
                        psums = [psum_main.tile([P, NB], f32, tag="ps", name=f"ps_v_{mt}_{i}") for i in range(CB)]
                        for cc in range(CC):
                            lhsT = x_t[cc][mt // 4][:, (mt % 4) * P : (mt % 4 + 1) * P]
                            for cb in range(CB):
                                nc.tensor.matmul(
                                    psums[cb],
                                    lhsT,
                                    wv_t[cc][cb],
                                    start=(cc == 0),
                                    stop=(cc == CC - 1),
                                )
                        for cb in range(CB):
                            nc.scalar.activation(
                                out=v_sb[:, mt, cb * NB : (cb + 1) * NB],
                                in_=psums[cb],
                                func=mybir.ActivationFunctionType.Copy,
                            )

                # ---- phase B: attention, one 512-wide n-block at a time ----
                with (
                    tc.tile_pool(name="attn", bufs=1) as attn_pool,
                    tc.tile_pool(name="pT", bufs=1) as pT_pool,
                    tc.tile_pool(name="small", bufs=8) as small_pool,
                    tc.tile_pool(name="outbuf", bufs=4) as out_pool,
                ):
                    o_sb = attn_pool.tile([P, MT, C], bf16, tag="o")

                    def emit_proj(s):
                        # out[2t+s, d] = sum_c2 o[1024s+c2, t] wprojT[c2, d]
                        for tt in range(CC):  # 8 t-tiles of 128 (t in [0,1024))
                            psums = [psum_main.tile([P, NB], f32, tag="ps", name=f"ps_p_{s}_{tt}_{i}") for i in range(CB)]
                            for k in range(CC):
                                lhsT = o_sb[:, CC * s + k, tt * P : (tt + 1) * P]
                                for db in range(CB):
                                    nc.tensor.matmul(
                                        psums[db],
                                        lhsT,
                                        wprojT_sb[:, k, db * NB : (db + 1) * NB],
                                        start=(k == 0),
                                        stop=(k == CC - 1),
                                    )
                            for db in range(CB):
                                outt = out_pool.tile([P, NB], f32, tag="outt", name=f"outt_{s}_{tt}_{db}")
                                nc.vector.tensor_add(
                                    out=outt,
                                    in0=psums[db],
                                    in1=bias_sb[:, db * NB : (db + 1) * NB],
                                )
                                nc.sync.dma_start(
                                    out=out_r[
                                        tt * P : (tt + 1) * P, s, db * NB : (db + 1) * NB
                                    ],
                                    in_=outt,
                                )

                    SB = 2  # n-blocks per superblock: one aT weight load
                    #         (z m-slice) feeds SB matmuls
                    for sbk in range(NBLK // SB):
                        pT = pT_pool.tile([P, MT, SB * NB], bf16, tag="pT")
                        # aT[m-tile, nblk] = z[:, m].T @ xT[:, nblk]; p = exp(aT/32)
                        for mt in range(MT):
                            apsums = [
                                psum_main.tile([P, NB], f32, tag="ps", name=f"ps_a_{sbk}_{mt}_{u}")
                                for u in range(SB)
                            ]
                            for cc in range(CC):
                                lhsT = z_sb[:, cc, mt * P : (mt + 1) * P]
                                for u in range(SB):
                                    nc.tensor.matmul(
                                        apsums[u],
                                        lhsT,
                                        x_t[cc][sbk * SB + u],
                                        start=(cc == 0),
                                        stop=(cc == CC - 1),
                                    )
                            for u in range(SB):
                                nc.scalar.activation(
                                    out=pT[:, mt, u * NB : (u + 1) * NB],
                                    in_=apsums[u],
                                    func=mybir.ActivationFunctionType.Exp,
                                    scale=SCALE,
                                )
                        # o[n-tile, c] = p[:, n].T @ v  (+ ones column -> rowsum)
                        for j in range(SB * NB // P):
                            nt = sbk * (SB * NB // P) + j
                            opsums = [psum_main.tile([P, NB], f32, tag="ps", name=f"ps_o_{nt}_{i}") for i in range(CB)]
                            osum = psum_sum.tile([P, 1], f32, tag="ps_sum", name=f"ps_sum_{nt}")
                            for mt in range(MT):
                                lhsT = pT[:, mt, j * P : (j + 1) * P]
                                for cb in range(CB):
                                    nc.tensor.matmul(
                                        opsums[cb],
                                        lhsT,
                                        v_sb[:, mt, cb * NB : (cb + 1) * NB],
                                        start=(mt == 0),
                                        stop=(mt == MT - 1),
                                    )
                                nc.tensor.matmul(
                                    osum,
                                    lhsT,
                                    v_sb[:, mt, C : C + 1],
                                    start=(mt == 0),
                                    stop=(mt == MT - 1),
                                )
                            recip = small_pool.tile([P, 1], f32, tag="recip")
                            nc.vector.reciprocal(out=recip, in_=osum)
                            for cb in range(CB):
                                nc.vector.tensor_scalar_mul(
                                    out=o_sb[:, nt, cb * NB : (cb + 1) * NB],
                                    in0=opsums[cb],
                                    scalar1=recip,
                                )
                        # phase C half s=sbk: its o-tiles (nt 0..7 for s=0,
                        # 8..15 for s=1) are exactly this superblock's output,
                        # so the proj matmuls + output DMAs interleave here.
                        emit_proj(sbk)
    if split_waits:
        _split_excess_waits(nc)
    return nc


_CACHED_NC = None


def _get_nc():
    global _CACHED_NC
    if _CACHED_NC is None:
        _CACHED_NC = build_nc()
    return _CACHED_NC


def _make_in_maps(x, w_qkv, w_proj, b_proj):
    bf16 = ml_dtypes.bfloat16
    x = np.asarray(x, dtype=np.float32)
    w_qkv = np.asarray(w_qkv, dtype=np.float32)
    w_proj = np.asarray(w_proj, dtype=np.float32)
    b_proj = np.asarray(b_proj, dtype=np.float32)

    w_q, w_k, w_v = w_qkv[0:C], w_qkv[C : 2 * C], w_qkv[2 * C : 3 * C]
    # hT = (W_q^T W_k)^T = W_k^T W_q, computed in f32 then rounded once
    hT = np.ascontiguousarray(w_k.T @ w_q).astype(bf16)
    wvT = np.ascontiguousarray(w_v.T).astype(bf16)
    wprojT = np.ascontiguousarray(w_proj.T).astype(bf16)
    bias = np.ascontiguousarray(np.broadcast_to(b_proj, (P, C)))
    in_maps = []
    for b in range(B):
        xT = np.ascontiguousarray(x[b].T).astype(bf16)
        in_maps.append(
            {"xT": xT, "hT": hT, "wvT": wvT, "wprojT": wprojT, "bias": bias}
        )
    return in_maps


def kernel(x, w_qkv, w_proj, b_proj):
    from concourse.bass_utils import run_bass_kernel_spmd

    nc = _get_nc()
    in_maps = _make_in_maps(x, w_qkv, w_proj, b_proj)
    res = run_bass_kernel_spmd(nc, in_maps, core_ids=list(range(B)))
    return np.stack([res.results[b]["out"] for b in range(B)], axis=0)


def kernel_traced(x, w_qkv, w_proj, b_proj, **trace_kwargs):
    """Like kernel() but with NTFF profiling; returns (out, BassKernelResults)."""
    from concourse.bass_utils import run_bass_kernel_spmd

    nc = _get_nc()
    in_maps = _make_in_maps(x, w_qkv, w_proj, b_proj)
    res = run_bass_kernel_spmd(
        nc, in_maps, core_ids=list(range(B)), trace=True, **trace_kwargs
    )
    out = np.stack([res.results[b]["out"] for b in range(B)], axis=0)
    return out, res


# revision 32
# speedup vs baseline: 1.1924x; 1.0012x over previous
"""Bass/Trainium2 kernel for the single-head dense attention block.

Reference computation (per batch element b of 8):
    qkv = x @ w_qkv.T                      # [N, 3C]
    q, k, v = qkv split                    # each [N, C]
    a = softmax(q @ k.T / sqrt(C))         # [N, N]
    o = a @ v                              # [N, C]
    o2 = o.swapaxes(0,1).reshape(N, C)     # torch-faithful permutation
    out = o2 @ w_proj.T + b_proj           # [N, C]

Sharding: batch B=8 data-parallel across the 8 NeuronCores, no collectives.

Layout strategy (zero on-device transposes; host pre-transposes weights/x):
  - q.k fold:  a[n,m] = x_n^T (W_q^T W_k) x_m, so the device never computes
    q or k. Host passes hT = (W_q^T W_k)^T = W_k^T W_q in bf16; the device
    computes z = hT.T @ xT  ([c,m] layout, 1/3 the cost of qT+kT), then
    aT[m,n] = z[:,m].T @ xT[:,n].
  - v computed in [m,c] layout:      v[m,c]  = xT[:,m].T @ wvT
  - p = exp(aT/32) (no max-sub; logits are O(5) so fp32 exp is safe)
  - o in [n,c] layout:               o[n,c]  = p[:,n].T @ v
    with v augmented by a ones column so rowsum(p) lands in [n,1] per-partition
  - the torch permutation satisfies out[2t+s, d] = sum_c2 o[1024s+c2, t] *
    wprojT[c2, d], i.e. proj is a plain matmul over o's partition axis in
    half-blocks; output rows written with a stride-2 row DMA.
"""

import numpy as np
import ml_dtypes

B, N, C = 8, 2048, 1024
P = 128
NB = 512          # free-dim block for matmuls (one PSUM bank)
SCALE = 1.0 / 32.0


def _patch_tile_drain():
    """Walrus in this container rejects >~4 sem waits on one instruction; the
    TileContext exit drain aggregates one wait per active processor. Re-emit
    them as individual SP wait_ge instructions before the drain."""
    import concourse.tile as tile
    from concourse import mybir
    from concourse.vector_clock import ScopedClock

    if getattr(tile.TileContext, "_drain_patched", False):
        return

    def _drain_and_barrier(self, tick_clock, wait_clock):
        nc = self.nc
        probe = nc.sync.nop(nofuse=True)
        wait_clock.add_sem_waits(
            probe.ins, ScopedClock({None: tick_clock.global_clock})
        )
        si = probe.ins.sync_info
        waits = list(si.on_wait) if si is not None and si.on_wait else []
        probe.ins.sync_info = mybir.SyncInfo(
            on_wait=[],
            on_update=list(si.on_update) if si is not None and si.on_update else [],
        )
        handles = {h.num: h for h in self.sems.allocated().values()}
        for w in waits:
            assert w.wait_mode == "sem-ge-imm", w
            nc.sync.wait_ge(handles[w.id], w.wait_value)
        nc.sync.drain()
        nc.all_engine_barrier()
        popped = nc._tile_sem_poison_stack.pop()
        assert popped is self._sem_poison
        nc.clear_and_free_semaphores(list(self.sems.allocated().values()))
        nc.all_engine_barrier()

    tile.TileContext._drain_and_barrier = _drain_and_barrier
    tile.TileContext._drain_patched = True


def _split_excess_waits(nc, max_keep=1):
    """Walrus in this container rejects instructions with more than a couple
    of sem waits. Move excess waits onto single-wait EventSemaphore
    instructions inserted just before the offender on the same engine
    (engines execute their stream in order, so a chain of waits == one
    multi-wait)."""
    from concourse import mybir

    ctr = 0
    for f in nc.m.functions:
        for bb in f.blocks:
            il = list(bb.instructions)
            out = []
            changed = False
            for inst in il:
                si = inst.sync_info
                waits = list(si.on_wait) if si is not None and si.on_wait else []
                if len(waits) > max_keep:
                    changed = True
                    excess, keep = waits[:-max_keep], waits[-max_keep:]
                    for w in excess:
                        ev = mybir.InstEventSemaphore(
                            name=f"I-wsplit-{ctr}", ins=[], outs=[]
                        )
                        ctr += 1
                        ev.engine = inst.engine
                        ev.sync_info = mybir.SyncInfo(on_wait=[w], on_update=[])
                        out.append(ev)
                    inst.sync_info = mybir.SyncInfo(
                        on_wait=keep,
                        on_update=list(si.on_update) if si.on_update else [],
                    )
                out.append(inst)
            if changed:
                bb.instructions = out
    return nc


def build_nc(split_waits=True):
    import concourse.bass as bass
    import concourse.tile as tile
    from concourse import mybir

    _patch_tile_drain()

    bf16 = mybir.dt.bfloat16
    f32 = mybir.dt.float32

    nc = bass.Bass()
    xT_ext = nc.declare_dram_parameter("xT", [C, N], bf16, isOutput=False)
    hT_ext = nc.declare_dram_parameter("hT", [C, C], bf16, isOutput=False)
    wvT_ext = nc.declare_dram_parameter("wvT", [C, C], bf16, isOutput=False)
    wprojT_ext = nc.declare_dram_parameter("wprojT", [C, C], bf16, isOutput=False)
    bias_ext = nc.declare_dram_parameter("bias", [P, C], f32, isOutput=False)
    out_ext = nc.declare_dram_parameter("out", [N, C], f32, isOutput=True)

    CC = C // P           # 8 contraction chunks over C
    MT = N // P           # 16 m-tiles
    NBLK = N // NB        # 4 n blocks
    CB = C // NB          # 2 c blocks

    xT_r = xT_ext[:, :].rearrange("(cc p) n -> p cc n", p=P)
    hT_r = hT_ext[:, :].rearrange("(cc p) d -> p cc d", p=P)
    wvT_r = wvT_ext[:, :].rearrange("(cc p) d -> p cc d", p=P)
    wprojT_r = wprojT_ext[:, :].rearrange("(cc p) d -> p cc d", p=P)
    out_r = out_ext[:, :].rearrange("(t s) d -> t s d", s=2)

    with tile.TileContext(nc) as tc:
        with (
            tc.tile_pool(name="persist", bufs=1) as persist,
            tc.tile_pool(name="psum_main", bufs=6, space="PSUM") as psum_main,
            tc.tile_pool(name="psum_sum", bufs=2, space="PSUM") as psum_sum,
        ):
            # ---- persistent SBUF tensors ----
            z_sb = persist.tile([P, CC, N], bf16, tag="z")
            v_sb = persist.tile([P, MT, C + 1], bf16, tag="v")
            wprojT_sb = persist.tile([P, CC, C], bf16, tag="wprojT")
            bias_sb = persist.tile([P, C], f32, tag="bias")

            # ones column for the softmax denominator
            nc.vector.memset(v_sb[:, :, C : C + 1], 1.0)

            # xT stays resident through phase B (aT rhs); weights pool is
            # freed after phase A. One [P, CC, NB] tile == one dma_start
            # (HWDGE fans a single transfer across rings; many small
            # dma_starts serialize ~0.55us each on the issuing sequencer).
            with tc.tile_pool(name="xpool", bufs=1) as xpool:
                x_sb = [
                    xpool.tile([P, CC, NB], bf16, tag=f"xsb_{nb}", name=f"xsb_{nb}")
                    for nb in range(NBLK)
                ]

                # ---- phase A: z = hT.T @ xT and v = xT.T @ wvT ----
                with tc.tile_pool(name="wpool", bufs=1) as wpool:
                    h_sb = [
                        wpool.tile([P, CC, NB], bf16, tag=f"hsb_{k}", name=f"hsb_{k}")
                        for k in range(CB)
                    ]
                    wv_sb = [
                        wpool.tile([P, CC, NB], bf16, tag=f"wvsb_{k}", name=f"wvsb_{k}")
                        for k in range(CB)
                    ]

                    # DMA order == consumption order; alternate SP/ACT rings.
                    nc.sync.dma_start(
                        out=h_sb[0], in_=hT_r[:, :, 0:NB]
                    )
                    nc.scalar.dma_start(out=x_sb[0], in_=xT_r[:, :, 0:NB])
                    nc.sync.dma_start(out=x_sb[1], in_=xT_r[:, :, NB : 2 * NB])
                    nc.scalar.dma_start(out=x_sb[2], in_=xT_r[:, :, 2 * NB : 3 * NB])
                    nc.sync.dma_start(out=x_sb[3], in_=xT_r[:, :, 3 * NB : 4 * NB])
                    nc.scalar.dma_start(out=h_sb[1], in_=hT_r[:, :, NB : 2 * NB])
                    nc.sync.dma_start(out=wv_sb[0], in_=wvT_r[:, :, 0:NB])
                    nc.scalar.dma_start(out=wv_sb[1], in_=wvT_r[:, :, NB : 2 * NB])
                    # needed only at the tail — load after the critical inputs
                    nc.sync.dma_start(out=bias_sb, in_=bias_ext[:, :])
                    nc.sync.dma_start(out=wprojT_sb, in_=wprojT_r)

                    # z[g-tile, n-block] = hT[:, g].T @ xT[:, n]
                    for j in range(CC):
                        psums = [psum_main.tile([P, NB], f32, tag="ps", name=f"ps_z_{j}_{i}") for i in range(NBLK)]
                        for cc in range(CC):
                            lhsT = h_sb[j // 4][:, cc, (j % 4) * P : (j % 4 + 1) * P]
                            for nb in range(NBLK):
                                nc.tensor.matmul(
                                    psums[nb],
                                    lhsT,
                                    x_sb[nb][:, cc, :],
                                    start=(cc == 0),
                                    stop=(cc == CC - 1),
                                )
                        for nb in range(NBLK):
                            nc.vector.tensor_copy(
                                out=z_sb[:, j, nb * NB : (nb + 1) * NB], in_=psums[nb]
                            )

                    # v[m-tile, c-block] = xT[:, m].T @ wvT[:, c]
                    for mt in range(MT):
                        psums = [psum_main.tile([P, NB], f32, tag="ps", name=f"ps_v_{mt}_{i}") for i in range(CB)]
                        for cc in range(CC):
                            lhsT = x_sb[mt // 4][:, cc, (mt % 4) * P : (mt % 4 + 1) * P]
                            for cb in range(CB):
                                nc.tensor.matmul(
                                    psums[cb],
                                    lhsT,
                                    wv_sb[cb][:, cc, :],
                                    start=(cc == 0),
                                    stop=(cc == CC - 1),
                                )
                        for cb in range(CB):
                            nc.scalar.activation(
                                out=v_sb[:, mt, cb * NB : (cb + 1) * NB],
                                in_=psums[cb],
                                func=mybir.ActivationFunctionType.Copy,
                            )

                # ---- phase B: attention, one 512-wide n-block at a time ----
                with (
                    tc.tile_pool(name="attn", bufs=1) as attn_pool,
                    tc.tile_pool(name="pT", bufs=1) as pT_pool,
                    tc.tile_pool(name="small", bufs=8) as small_pool,
                    tc.tile_pool(name="outbuf", bufs=4) as out_pool,
                ):
                    o_sb = attn_pool.tile([P, MT, C], bf16, tag="o")

                    def emit_proj(s):
                        # out[2t+s, d] = sum_c2 o[1024s+c2, t] wprojT[c2, d]
                        for tt in range(CC):  # 8 t-tiles of 128 (t in [0,1024))
                            psums = [psum_main.tile([P, NB], f32, tag="ps", name=f"ps_p_{s}_{tt}_{i}") for i in range(CB)]
                            for k in range(CC):
                                lhsT = o_sb[:, CC * s + k, tt * P : (tt + 1) * P]
                                for db in range(CB):
                                    nc.tensor.matmul(
                                        psums[db],
                                        lhsT,
                                        wprojT_sb[:, k, db * NB : (db + 1) * NB],
                                        start=(k == 0),
                                        stop=(k == CC - 1),
                                    )
                            for db in range(CB):
                                outt = out_pool.tile([P, NB], f32, tag="outt", name=f"outt_{s}_{tt}_{db}")
                                nc.vector.tensor_add(
                                    out=outt,
                                    in0=psums[db],
                                    in1=bias_sb[:, db * NB : (db + 1) * NB],
                                )
                                nc.sync.dma_start(
                                    out=out_r[
                                        tt * P : (tt + 1) * P, s, db * NB : (db + 1) * NB
                                    ],
                                    in_=outt,
                                )

                    SB = 2  # n-blocks per superblock: one aT weight load
                    #         (z m-slice) feeds SB matmuls
                    for sbk in range(NBLK // SB):
                        pT = pT_pool.tile([P, MT, SB * NB], bf16, tag="pT")
                        # aT[m-tile, nblk] = z[:, m].T @ xT[:, nblk]; p = exp(aT/32)
                        for mt in range(MT):
                            apsums = [
                                psum_main.tile([P, NB], f32, tag="ps", name=f"ps_a_{sbk}_{mt}_{u}")
                                for u in range(SB)
                            ]
                            for cc in range(CC):
                                lhsT = z_sb[:, cc, mt * P : (mt + 1) * P]
                                for u in range(SB):
                                    nc.tensor.matmul(
                                        apsums[u],
                                        lhsT,
                                        x_sb[sbk * SB + u][:, cc, :],
                                        start=(cc == 0),
                                        stop=(cc == CC - 1),
                                    )
                            for u in range(SB):
                                nc.scalar.activation(
                                    out=pT[:, mt, u * NB : (u + 1) * NB],
                                    in_=apsums[u],
                                    func=mybir.ActivationFunctionType.Exp,
                                    scale=SCALE,
                                )
                        # o[n-tile, c] = p[:, n].T @ v  (+ ones column -> rowsum)
                        for j in range(SB * NB // P):
                            nt = sbk * (SB * NB // P) + j
                            opsums = [psum_main.tile([P, NB], f32, tag="ps", name=f"ps_o_{nt}_{i}") for i in range(CB)]
                            osum = psum_sum.tile([P, 1], f32, tag="ps_sum", name=f"ps_sum_{nt}")
                            for mt in range(MT):
                                lhsT = pT[:, mt, j * P : (j + 1) * P]
                                for cb in range(CB):
                                    nc.tensor.matmul(
                                        opsums[cb],
                                        lhsT,
                                        v_sb[:, mt, cb * NB : (cb + 1) * NB],
                                        start=(mt == 0),
                                        stop=(mt == MT - 1),
                                    )
                                nc.tensor.matmul(
                                    osum,
                                    lhsT,
                                    v_sb[:, mt, C : C + 1],
                                    start=(mt == 0),
                                    stop=(mt == MT - 1),
                                )
                            recip = small_pool.tile([P, 1], f32, tag="recip")
                            nc.vector.reciprocal(out=recip, in_=osum)
                            for cb in range(CB):
                                nc.vector.tensor_scalar_mul(
                                    out=o_sb[:, nt, cb * NB : (cb + 1) * NB],
                                    in0=opsums[cb],
                                    scalar1=recip,
                                )
                        # phase C half s=sbk: its o-tiles (nt 0..7 for s=0,
                        # 8..15 for s=1) are exactly this superblock's output,
                        # so the proj matmuls + output DMAs interleave here.
                        emit_proj(sbk)
    if split_waits:
        _split_excess_waits(nc)
    return nc


_CACHED_NC = None


def _get_nc():
    global _CACHED_NC
    if _CACHED_NC is None:
        _CACHED_NC = build_nc()
    return _CACHED_NC


def _make_in_maps(x, w_qkv, w_proj, b_proj):
    bf16 = ml_dtypes.bfloat16
    x = np.asarray(x, dtype=np.float32)
    w_qkv = np.asarray(w_qkv, dtype=np.float32)
    w_proj = np.asarray(w_proj, dtype=np.float32)
    b_proj = np.asarray(b_proj, dtype=np.float32)

    w_q, w_k, w_v = w_qkv[0:C], w_qkv[C : 2 * C], w_qkv[2 * C : 3 * C]
    # hT = (W_q^T W_k)^T = W_k^T W_q, computed in f32 then rounded once
    hT = np.ascontiguousarray(w_k.T @ w_q).astype(bf16)
    wvT = np.ascontiguousarray(w_v.T).astype(bf16)
    wprojT = np.ascontiguousarray(w_proj.T).astype(bf16)
    bias = np.ascontiguousarray(np.broadcast_to(b_proj, (P, C)))
    in_maps = []
    for b in range(B):
        xT = np.ascontiguousarray(x[b].T).astype(bf16)
        in_maps.append(
            {"xT": xT, "hT": hT, "wvT": wvT, "wprojT": wprojT, "bias": bias}
        )
    return in_maps


def kernel(x, w_qkv, w_proj, b_proj):
    from concourse.bass_utils import run_bass_kernel_spmd

    nc = _get_nc()
    in_maps = _make_in_maps(x, w_qkv, w_proj, b_proj)
    res = run_bass_kernel_spmd(nc, in_maps, core_ids=list(range(B)))
    return np.stack([res.results[b]["out"] for b in range(B)], axis=0)


def kernel_traced(x, w_qkv, w_proj, b_proj, **trace_kwargs):
    """Like kernel() but with NTFF profiling; returns (out, BassKernelResults)."""
    from concourse.bass_utils import run_bass_kernel_spmd

    nc = _get_nc()
    in_maps = _make_in_maps(x, w_qkv, w_proj, b_proj)
    res = run_bass_kernel_spmd(
        nc, in_maps, core_ids=list(range(B)), trace=True, **trace_kwargs
    )
    out = np.stack([res.results[b]["out"] for b in range(B)], axis=0)
    return out, res
